# revision 41
# baseline (speedup 1.0000x reference)
"""HardClusterAssigner Trainium2 kernel.

Reference computation:
    x_emb = mean_b(einsum('bsv,hs->bvh', x, W) + b)   # [V, H]
    assignments = one_hot(argmin(-l2norm(x_emb) @ l2norm(centroids).T))

Key transformations:
  1. mean over B commutes with the linear contraction over S, the l2norm of
     the embedding is a positive per-row scale (argmin-invariant), and the
     1/B + bias fold in exactly:
         sim[v,c] = (sum_b x)[s,v] @ M[s,c] + bn[c],
         M = W.T @ cn.T,  bn = B * (b @ cn.T),  cn = l2norm(centroids)
     M/bn are x-independent and folded on the host (fp64); M ships as an
     exact fp16 hi+lo pair (~1e-7 relative), bn as an fp32 column appended
     to the identity constant and added per-partition on the DVE.
  2. x streams as fp16 (halves the dominant HBM traffic) in [s, b, v]
     layout as 16 half-chunk tiles on the SP HWDGE ring (a second ring
     steals SDMA packet slots, so everything rides one ring). Each tile's
     b-reduction runs as a halving add tree on the DVE over fully
     contiguous 2D slices (fp16 2x packed mode) — 1 level for the middle
     chunks (keeps the DVE under the stream rate), 2 for the edge chunks
     (light PE at the cold head and short tail chain). The PE contracts
     the remaining (s, slab) axes with fp16xfp16 products accumulated
     exactly in fp32 PSUM (M hi/lo stationary); slab segments beyond the
     first overlay the same PSUM columns, so one bank serves all tree
     depths. Verified argmax-exact on the reference inputs with a 1.8e-3
     worst-row margin (~90x the device-vs-host numeric noise).
  3. sim lands PSUM-transposed as [c, (slab v)]; a slab-reduce + identity-
     matmul transpose (exact: multiplies by 1.0/0.0) accumulates [v, c]
     into PSUM for the row-max + is_equal one-hot.

Sharding: V is split across the 8 cores; every stage after the split is
core-local (no collectives). Per-core time is DMA-bound: ~8.9 MB/core
(x 8.4 MB fp16 + M 0.26 MB) at the ~358 GB/s HBM roofline, with the DVE
trees (~17us) and all PE work hidden under the stream.
"""

import sys

for _p in ("/opt/trn_rl_repo",):
    if _p not in sys.path:
        sys.path.append(_p)

from contextlib import ExitStack

import numpy as np

import concourse.bacc as bacc
import concourse.bass as bass
import concourse.mybir as mybir
from concourse import tile
from concourse.bass_utils import run_bass_kernel_spmd

B, S, V, H, C = 64, 1024, 512, 512, 64
NCORES = 8
VL = V // NCORES  # 64 V-columns per core
P = 128
ST = S // P  # 8 s-chunks
FH = (B // 2) * VL  # 2048 free elems per half-chunk tile
JH = 8  # psC slab count; deeper trees hit it exactly, shallower overlay it
F16 = mybir.dt.float16
F32 = mybir.dt.float32

_NC_CACHE = None


def build_bass() -> bass.Bass:
    nc = bacc.Bacc("TRN2", target_bir_lowering=False)

    xs = nc.declare_dram_parameter("xs", [S, 2 * FH], F16, isOutput=False)
    mm = nc.declare_dram_parameter("mm", [P, 2 * ST * C], F16, isOutput=False)
    c32 = nc.declare_dram_parameter("c32", [C, C + 1], F32, isOutput=False)
    out = nc.declare_dram_parameter("out", [VL, C], F32, isOutput=True)

    with tile.TileContext(nc) as tc, ExitStack() as ctx:
        consts = ctx.enter_context(tc.tile_pool(name="consts", bufs=1))
        xhpool = ctx.enter_context(tc.tile_pool(name="xh", bufs=16))
        spool = ctx.enter_context(tc.tile_pool(name="small", bufs=1))
        pst = ctx.enter_context(tc.tile_pool(name="pst", bufs=1, space="PSUM"))
        psca = ctx.enter_context(tc.tile_pool(name="psca", bufs=1, space="PSUM"))

        # const tiles; their DMAs ride the SP ring just behind the first
        # chunk (a second ring steals SDMA packet slots and slows the
        # stream). Flat 2D transfers only.
        msb = consts.tile([P, 2 * ST * C], F16)
        idt = consts.tile([C, C + 1], F32)  # identity | bn column (fp32)

        # final sim [v, c]; single PE-side accumulator [c, (slab v)] — both
        # b-halves overlay the same slab columns (their sums just add)
        psT = pst.tile([VL, C], F32, tag="psT")
        psC = psca.tile([C, JH * VL], F32, tag="psC")

        xs_r = xs.rearrange("(t p) f -> t p f", p=P)
        for t in range(ST):
            # middle chunks run 1-level trees (16 slabs, extra PE segment)
            # to keep the DVE well under the stream rate; edge chunks run
            # 2-level (light PE while cold at the head, short tail chain)
            nlvl = 1 if (2 <= t <= 5 or t == ST - 1) else 2
            nun, width, pool = 2, FH, xhpool
            xhs = []
            for h in range(nun):
                xv = pool.tile([P, width], F16, tag="xh")
                nc.sync.dma_start(
                    out=xv[:], in_=xs_r[t][:, h * width : (h + 1) * width]
                )
                # halving add tree over contiguous column blocks (fp16 2x
                # mode); cols = b_local*VL + v
                nb = width
                for _ in range(nlvl):
                    hb = nb // 2
                    nc.vector.tensor_tensor(
                        xv[:, 0:hb], xv[:, 0:hb], xv[:, hb:nb],
                        op=mybir.AluOpType.add,
                    )
                    nb = hb
                xhs.append((xv, 0, nb))
            if t == 0:
                # consts land behind chunk 0 on the SP ring (a second ring
                # steals SDMA packet slots); the PE has slack to wait
                nc.sync.dma_start(out=msb[:], in_=mm[:])
                nc.sync.dma_start(out=idt[:], in_=c32[:])
            # slab contraction, M_t hi/lo stationary; slab segments beyond
            # the first overlay the same psC columns (sums just accumulate)
            for h in range(nun):
                xv, base, nb = xhs[h]
                for li in range(2):
                    nseg = nb // (JH * VL)
                    for g in range(nseg):
                        nc.tensor.matmul(
                            psC[:],
                            msb[:, (li * ST + t) * C : (li * ST + t + 1) * C],
                            xv[:, base + g * JH * VL : base + (g + 1) * JH * VL],
                            start=(t == 0 and h == 0 and li == 0 and g == 0),
                            stop=(
                                t == ST - 1 and h == nun - 1 and li == 1
                                and g == nseg - 1
                            ),
                        )

        # --- tail: slab-reduce, transpose into [v, c], argmax --------------
        sC = spool.tile([C, VL], F32, tag="sC")
        nc.vector.tensor_reduce(
            sC[:],
            psC[:].rearrange("c (s v) -> c v s", s=JH),
            axis=mybir.AxisListType.X,
            op=mybir.AluOpType.add,
        )
        nc.vector.tensor_scalar(
            sC[:], sC[:], idt[:, C : C + 1], None, op0=mybir.AluOpType.add
        )
        nc.tensor.matmul(psT[:], sC[:], idt[:, 0:C], start=True, stop=True)

        mx = spool.tile([VL, 1], F32)
        nc.vector.tensor_reduce(
            mx[:], psT[:], axis=mybir.AxisListType.X, op=mybir.AluOpType.max
        )
        oh = spool.tile([VL, C], F32)
        nc.vector.tensor_scalar(
            oh[:], psT[:], mx[:], None, op0=mybir.AluOpType.is_equal
        )
        nc.sync.dma_start(out=out[:], in_=oh[:])

    nc.compile()
    return nc


def _get_nc() -> bass.Bass:
    global _NC_CACHE
    if _NC_CACHE is None:
        _NC_CACHE = build_bass()
    return _NC_CACHE


def make_in_maps(x, W, b, centroids):
    x = np.asarray(x, dtype=np.float32)
    W = np.asarray(W, dtype=np.float32)
    b = np.asarray(b, dtype=np.float32)
    centroids = np.asarray(centroids, dtype=np.float32)

    # x-independent folds, in float64, shipped as exact fp16 hi+lo pairs
    cn = centroids.astype(np.float64)
    cn /= np.linalg.norm(cn, axis=1, keepdims=True)
    M = W.astype(np.float64).T @ cn.T  # [S, C]
    bn = np.float64(B) * (b.astype(np.float64) @ cn.T)  # [C]

    Mhi = M.astype(np.float16)
    Mlo = (M - Mhi.astype(np.float64)).astype(np.float16)
    mhost = np.empty((P, 2, ST, C), np.float16)
    mhost[:, 0] = Mhi.reshape(ST, P, C).transpose(1, 0, 2)
    mhost[:, 1] = Mlo.reshape(ST, P, C).transpose(1, 0, 2)
    mhost = np.ascontiguousarray(mhost).reshape(P, 2 * ST * C)

    c32host = np.concatenate(
        [np.eye(C, dtype=np.float32), bn.astype(np.float32)[:, None]], axis=1
    )

    # Host layout [B,S,V] -> [S, B, VL] per core, in fp16 (cast first so the
    # transpose moves half the bytes). One pass to [S, B, V] (contiguous 1KB
    # runs), then a contiguous per-core V-slice.
    x16 = x.astype(np.float16)
    xsb = np.ascontiguousarray(x16.transpose(1, 0, 2))  # [S, B, V]
    in_maps = []
    for i in range(NCORES):
        xs_i = np.ascontiguousarray(
            xsb[:, :, i * VL : (i + 1) * VL]
        ).reshape(S, 2 * FH)
        in_maps.append(
            {"xs": xs_i, "mm": mhost, "c32": c32host}
        )
    return in_maps


def run(inputs: dict, trace: bool = False):
    """Run on the 8 NeuronCores; returns (full_output, BassKernelResults)."""
    nc = _get_nc()
    in_maps = make_in_maps(**inputs)
    res = run_bass_kernel_spmd(nc, in_maps, list(range(NCORES)), trace=trace)
    full = np.concatenate([r["out"] for r in res.results], axis=0)
    return full, res


def kernel(x, W, b, centroids) -> np.ndarray:
    full, _ = run({"x": x, "W": W, "b": b, "centroids": centroids})
    return full
